# revision 37
# baseline (speedup 1.0000x reference)
"""Trainium2 Bass kernel for nn_BottomLevelDecoderRNN.

2-layer GRU decoder, H=1024, S=16 steps, E*B = 2048 independent sequences,
data-parallel over 8 NeuronCores (R = 256 rows per core), everything kept
transposed as [feature, row].

Per-step math (per core), PE work 360 passes (DR fp8 except n2/fco):
  A rz1:  ps = DR(wp8, pv8) + DR(w1h_rz, h1_8)  [x256 PSUM]
          +cached_rz (DVE) -> sigmoid(ps/256)            [80 DR]
  B/C n1: psh = DR(w1hn8, h1_8) [x256]; tt1 = (psh+256*bhh1n)*r1 (DVE stt)
          tt1 += cached_n; psg pair = DR(wp_n, pv8); psg += tt1
          -> tanh(psg/256) paired                        [32+8 DR]
  D gh2n: psh2 = DR(w2hn8, h2_8) [x256]; ghb2 = psh2 + 256*bhh2n (DVE)
                                                         [32 DR]
  E fco(s-1) fp16                                        [16 fp16]
  F rz2:  ps = DR(w2h_rz, h2_8) + DR(w2i_rz, h1'_8) -> sigmoid(ps/256+b)
          tt2 = r2*ghb2 (DVE 4-wide)                     [128 DR]
  G n2:   psg2 = w2in16[x256] @ h1' (fp16); psg2 += tt2
          -> tanh(psg2/256 + bih2n)                      [64 fp16]

fp8 path: weights scaled x256 into e4m3 (clip 240); h states / prev cast to
e4m3 unscaled. w2in (error-sensitive) stays fp16 (x256-scaled values).
All weights SBUF-resident, DMAs spread over 4 queues. Emulated rel-err 1.39e-2.
"""
import numpy as np

E, B, C, H, D = 16, 128, 512, 1024, 130
S = 16
NCORES = 8
EPC = E // NCORES        # 2 embeddings per core
R = EPC * B              # 256 rows per core
KH = H // 128            # 8 h k-tiles
KP = KH // 2             # 4 DR k-pairs
MG = 3 * H // 128        # 24 gate m-tiles
MRZ = 2 * H // 128       # 16 rz m-tiles
NJ = H // 128            # 8 n/h tiles
KC = C // 128            # 4 c k-tiles
MI = 2 * H // 128        # 16 init m-tiles
WS = 256.0               # fp8 weight scale

# bias tile column layout ([128, NBIAS] fp32)
B_INIT = 0      # 16: fc_init_b
B_N1H = 32      # 8:  bhh1[2H:]*256
B_IH1 = 40      # 24: rz: (bih1+bhh1)*256;  n: bih1*256
B_RZ2 = 64      # 16: bih2[:2H]+bhh2[:2H]
B_N2H = 80      # 8:  bhh2[2H:]*256
B_N2I = 88      # 8:  bih2[2H:]
B_FCO = 96      # 2:  fco_b
NBIAS = 98

_cache = {}


def _wtiles(w_t, nm, nk):
    """[K, M] (w_t = W.T) -> [nm, 128, nk*128] fp16 stationary chunks."""
    Kf, Mf = w_t.shape
    assert Kf == nk * 128 and Mf == nm * 128
    return np.ascontiguousarray(
        w_t.reshape(nk, 128, nm, 128).transpose(2, 1, 0, 3).reshape(nm, 128, nk * 128)
    ).astype(np.float16)


def _q8(x):
    import ml_dtypes
    return np.clip(x, -240, 240).astype(ml_dtypes.float8_e4m3)


def _wtiles_dr(w, nm, scale=WS):
    """[nm*128, H] weight part -> [nm, 128, KP*2*128] fp8e4 DoubleRow chunks:
    chunk[m][p, kt, j, c] = (W.T)[kt*256 + j*128 + p, m*128 + c] * scale."""
    wt = np.asarray(w, np.float32).T * scale          # [H, nm*128]
    arr = wt.reshape(KP, 2, 128, nm, 128).transpose(3, 2, 0, 1, 4)
    return np.ascontiguousarray(_q8(arr).reshape(nm, 128, KP * 2 * 128))


def _bias_cols(vec, n):
    return np.ascontiguousarray(vec.reshape(n, 128).T).astype(np.float32)


def build_program():
    import concourse.tile as tile
    from concourse import bacc, mybir

    f32, f16, f8 = mybir.dt.float32, mybir.dt.float16, mybir.dt.float8e4
    Sig = mybir.ActivationFunctionType.Sigmoid
    Tanh = mybir.ActivationFunctionType.Tanh
    Ident = mybir.ActivationFunctionType.Identity
    DRow = mybir.MatmulPerfMode.DoubleRow
    ADD = mybir.AluOpType.add
    MULT = mybir.AluOpType.mult

    nc = bacc.Bacc("TRN2", target_bir_lowering=False, debug=False,
                   enable_asserts=False, num_devices=NCORES)

    def din(name, shape, dt=f16):
        return nc.dram_tensor(name, shape, dt, kind="ExternalInput").ap()

    cflatT = din("cflatT", [128, KC * R])
    prevT8 = din("prevT8", [S, 128, 2, R], f8)
    w1h8 = din("w1h8", [MRZ // 4, 128, 4 * KP * 2 * 128], f8)
    w2i8 = din("w2i8", [MRZ // 4, 128, 4 * KP * 2 * 128], f8)
    w2h8 = din("w2h8", [MRZ // 4, 128, 4 * KP * 2 * 128], f8)
    w2hn8 = din("w2hn8", [NJ // 4, 128, 4 * KP * 2 * 128], f8)
    w1hn8 = din("w1hn8", [NJ // 4, 128, 4 * KP * 2 * 128], f8)
    w2in = din("w2in", [NJ // 4, 128, 4 * KH * 128])
    wp8 = din("wp8", [128, MG, 2, 128], f8)
    wc = din("wc", [6, 128, 4 * KC * 128])      # 4 m-tiles per chunk
    wini = din("wini", [4, 128, 4 * KC * 128])
    wfco = din("wfco", [128, KH * 256])
    biases = din("biases", [128, NBIAS], f32)
    yT = nc.dram_tensor("yT", [S, 132, R], f32, kind="ExternalOutput").ap()

    with tile.TileContext(nc) as tc:
        with tc.tile_pool(name="const", bufs=1) as const, \
             tc.tile_pool(name="stream", bufs=7) as stream, \
             tc.tile_pool(name="state", bufs=2) as state, \
             tc.tile_pool(name="gates", bufs=2) as gates, \
             tc.tile_pool(name="tmp", bufs=2) as tmp, \
             tc.tile_pool(name="prevp", bufs=3) as prevp, \
             tc.tile_pool(name="outp", bufs=2) as outp, \
             tc.tile_pool(name="psA", bufs=3, space="PSUM") as psA, \
             tc.tile_pool(name="psB", bufs=5, space="PSUM") as psB:

            # ---- SBUF-resident tiles ----
            bias_sb = const.tile([128, NBIAS], f32, tag="bias")
            cfl_sb = const.tile([128, KC * R], f16, tag="cfl")
            w1h8_sb = const.tile([128, MRZ, KP, 2, 128], f8, tag="w1h8")
            w2i8_sb = const.tile([128, MRZ, KP, 2, 128], f8, tag="w2i8")
            w2h8_sb = const.tile([128, MRZ, KP, 2, 128], f8, tag="w2h8")
            w2hn8_sb = const.tile([128, NJ, KP, 2, 128], f8, tag="w2hn8")
            w1hn8_sb = const.tile([128, NJ, KP, 2, 128], f8, tag="w1hn8")
            w2in_sb = const.tile([128, NJ, KH * 128], f16, tag="w2in")
            wp8_sb = const.tile([128, MG, 2, 128], f8, tag="wp8")
            wfco_sb = const.tile([128, KH * 256], f16, tag="wfco")
            cached_sb = const.tile([128, MG, R], f16, tag="cached")

            # ---- stream-chunk tiles for init GEMMs (wini 4 + wc 6) ----
            wini_ch = [stream.tile([128, 4 * KC * 128], f16, tag="stream",
                                   name=f"wini_ch{i}") for i in range(4)]
            wc_ch = [stream.tile([128, 4 * KC * 128], f16, tag="stream",
                                 name=f"wc_ch{i}") for i in range(6)]

            # ---- all input DMAs up-front, spread over the 3 DMA-capable
            # queues (sync, scalar, gpsimd); per-queue order matches
            # first use ----
            qs_sync, qs_sc, qs_gp = nc.sync, nc.scalar, nc.gpsimd
            qs_sync.dma_start(cfl_sb[:], cflatT[:])
            qs_sc.dma_start(bias_sb[:], biases[:])
            qs_sync.dma_start(wini_ch[0][:], wini[0])
            qs_sc.dma_start(wini_ch[1][:], wini[1])
            qs_gp.dma_start(wini_ch[2][:], wini[2])
            qs_gp.dma_start(wini_ch[3][:], wini[3])
            qs_sync.dma_start(wc_ch[0][:], wc[0])
            qs_sc.dma_start(wc_ch[1][:], wc[1])
            qs_gp.dma_start(wc_ch[2][:], wc[2])
            qs_sync.dma_start(wc_ch[3][:], wc[3])
            # (wc4/wc5 feed cached_n, first used mid step 0 — issued last,
            # after the step-0 weights, so their stream-buffer waits can't
            # block the critical queue entries)
            # step-0 A/B/C weights
            qs_sc.dma_start(wp8_sb[:], wp8[:])
            for g in range(MRZ // 4):
                [qs_sync, qs_sc, qs_gp, qs_sync][g % 4].dma_start(
                    w1h8_sb[:, 4 * g:4 * g + 4], w1h8[g])
            for g in range(NJ // 4):
                [qs_gp, qs_sc][g % 2].dma_start(
                    w1hn8_sb[:, 4 * g:4 * g + 4], w1hn8[g])
            # step-0 D/F/G weights
            for g in range(NJ // 4):
                [qs_gp, qs_sync][g % 2].dma_start(
                    w2hn8_sb[:, 4 * g:4 * g + 4], w2hn8[g])
            for g in range(MRZ // 4):
                [qs_sync, qs_sc, qs_gp, qs_sync][g % 4].dma_start(
                    w2h8_sb[:, 4 * g:4 * g + 4], w2h8[g])
            for g in range(MRZ // 4):
                [qs_gp, qs_sc, qs_sc, qs_sync][g % 4].dma_start(
                    w2i8_sb[:, 4 * g:4 * g + 4], w2i8[g])
            for g in range(NJ // 4):
                [qs_sync, qs_gp][g % 2].dma_start(
                    w2in_sb[:, 4 * g:4 * g + 4], w2in[g])
            qs_sc.dma_start(wfco_sb[:], wfco[:])
            qs_sc.dma_start(wc_ch[4][:], wc[4])
            qs_gp.dma_start(wc_ch[5][:], wc[5])

            def bias_ap(col):
                return bias_sb[:, col:col + 1]

            # ---- h init: t0T = tanh(wini @ cflatT + binit) ----
            h1T = state.tile([128, NJ, R], f16, tag="h1")
            h2T = state.tile([128, NJ, R], f16, tag="h2")
            h18 = state.tile([128, NJ, R], f8, tag="h18")
            h28 = state.tile([128, NJ, R], f8, tag="h28")
            for g in range(4):
                wchunk = wini_ch[g]
                for mi in range(4):
                    m = 4 * g + mi
                    ps = psB.tile([128, R], f32, tag="g")
                    for k in range(KC):
                        nc.tensor.matmul(
                            ps[:], wchunk[:, (mi * KC + k) * 128:(mi * KC + k + 1) * 128],
                            cfl_sb[:, k * R:(k + 1) * R],
                            start=(k == 0), stop=(k == KC - 1))
                    dst = h1T if m < NJ else h2T
                    nc.scalar.activation(dst[:, m % NJ], ps[:], Tanh,
                                         bias=bias_ap(B_INIT + m))
            for j in range(NJ):
                nc.vector.tensor_copy(h18[:, j], h1T[:, j])
                nc.vector.tensor_copy(h28[:, j], h2T[:, j])

            # ---- cached = Wc @ cflatT + biases (scaled x256); rz chunks
            # (g 0-3) pre-loop, n chunks (g 4-5) deferred into step 0 ----
            def cached_chunk(g):
                wchunk = wc_ch[g]
                for mi in range(4):
                    m = 4 * g + mi
                    ps = psB.tile([128, R], f32, tag="g")
                    for k in range(KC):
                        nc.tensor.matmul(
                            ps[:], wchunk[:, (mi * KC + k) * 128:(mi * KC + k + 1) * 128],
                            cfl_sb[:, k * R:(k + 1) * R],
                            start=(k == 0), stop=(k == KC - 1))
                    nc.scalar.activation(cached_sb[:, m], ps[:], Ident,
                                         bias=bias_ap(B_IH1 + m), scale=WS)

            for g in range(4):
                cached_chunk(g)

            def fco_step(h2T_cur, s):
                for mo, msz, osz, bc in [(0, 128, 128, B_FCO), (128, 32, 2, B_FCO + 1)]:
                    ps = psB.tile([128, R], f32, tag="g")
                    for k in range(KH):
                        nc.tensor.matmul(ps[0:msz, :],
                                         wfco_sb[:, k * 256 + mo: k * 256 + mo + msz],
                                         h2T_cur[:, k],
                                         start=(k == 0), stop=(k == KH - 1))
                    ysb = outp.tile([128, R], f32, tag="y")
                    nc.scalar.activation(ysb[0:osz, :], ps[0:osz, :], Ident,
                                         bias=bias_sb[0:osz, bc:bc + 1])
                    nc.sync.dma_start(yT[s, mo:mo + osz, :], ysb[0:osz, :])

            h2T_done = []  # (h2T tile, step) pending fco

            for s in range(S):
                pv8 = prevp.tile([128, 2, R], f8, tag="pv8")
                nc.sync.dma_start(pv8[:], prevT8[s])

                # ---------- B-head: first half of GRU1-n h-part. Pure PE
                # work with no vector consumer yet — covers the vector tail
                # of the previous step's h2 update so A's adds aren't
                # delayed. ----------
                psh_t = {}
                for j in range(NJ // 2):
                    psh = psB.tile([128, R], f32, tag="g")
                    for kt in range(KP):
                        nc.tensor.matmul(psh[:], w1hn8_sb[:, j, kt],
                                         h18[:, 2 * kt:2 * kt + 2, :],
                                         start=(kt == 0), stop=(kt == KP - 1),
                                         perf_mode=DRow)
                    psh_t[j] = psh

                # ---------- A: GRU1 r/z (fp8 DR, x256 PSUM; m-tile pairs
                # share a [128,2,R] PSUM tile and one ACT) ----------
                r1 = gates.tile([128, NJ, R], f16, tag="rg")
                z1 = gates.tile([128, NJ, R], f16, tag="zg")
                for p in range(MRZ // 2):
                    ps = psA.tile([128, 2, R], f32, tag="rz")
                    for mi in range(2):
                        m = 2 * p + mi
                        nc.tensor.matmul(ps[:, mi], wp8_sb[:, m], pv8[:],
                                         start=True, stop=False, perf_mode=DRow)
                        for kt in range(KP):
                            nc.tensor.matmul(ps[:, mi], w1h8_sb[:, m, kt],
                                             h18[:, 2 * kt:2 * kt + 2, :],
                                             start=False, stop=(kt == KP - 1),
                                             perf_mode=DRow)
                    nc.vector.tensor_add(ps[:], ps[:], cached_sb[:, 2 * p:2 * p + 2])
                    dst = r1 if p < NJ // 2 else z1
                    jj = (2 * p) % NJ
                    nc.scalar.activation(dst[:, jj:jj + 2], ps[:], Sig,
                                         bias=0.0, scale=1.0 / WS)

                if s == 0:
                    cached_chunk(4)
                    cached_chunk(5)

                # ---------- B-tail/C: GRU1 n (all fp8 DR; x256 PSUM) -------
                # psh_j = w1hn8 @ h18; tt1_j = (psh + 256*bhh1n)*r1 (DVE stt)
                tt1 = tmp.tile([128, NJ, R], f16, tag="tt")
                for j in range(NJ):
                    if j in psh_t:
                        psh = psh_t.pop(j)
                    else:
                        psh = psB.tile([128, R], f32, tag="g")
                        for kt in range(KP):
                            nc.tensor.matmul(psh[:], w1hn8_sb[:, j, kt],
                                             h18[:, 2 * kt:2 * kt + 2, :],
                                             start=(kt == 0), stop=(kt == KP - 1),
                                             perf_mode=DRow)
                    nc.vector.scalar_tensor_tensor(
                        tt1[:, j], psh[:], bias_ap(B_N1H + j), r1[:, j],
                        op0=ADD, op1=MULT)
                    if j % 2 == 1:
                        hs = slice(j - 1, j + 1)
                        nc.gpsimd.tensor_add(tt1[:, hs], tt1[:, hs],
                                             cached_sb[:, MRZ + j - 1:MRZ + j + 1])
                n1 = gates.tile([128, NJ, R], f16, tag="ng")
                for p in range(NJ // 2):
                    psg = psA.tile([128, 2, R], f32, tag="rz")
                    for mi in range(2):
                        j = 2 * p + mi
                        nc.tensor.matmul(psg[:, mi], wp8_sb[:, MRZ + j], pv8[:],
                                         start=True, stop=True, perf_mode=DRow)
                    nc.vector.tensor_add(psg[:], psg[:], tt1[:, 2 * p:2 * p + 2])
                    nc.scalar.activation(n1[:, 2 * p:2 * p + 2], psg[:], Tanh,
                                         bias=0.0, scale=1.0 / WS)
                # h1' = n1 + z1*(h1 - n1) in j-halves; fp8 result first (it
                # gates F), fp16 copy after
                d1 = tmp.tile([128, NJ, R], f16, tag="tt")
                h1T_new = state.tile([128, NJ, R], f16, tag="h1")
                h18_new = state.tile([128, NJ, R], f8, tag="h18")
                for hf in range(2):
                    hs = slice(4 * hf, 4 * hf + 4)
                    nc.vector.tensor_sub(d1[:, hs], h1T[:, hs], n1[:, hs])
                    nc.vector.tensor_mul(d1[:, hs], z1[:, hs], d1[:, hs])
                    nc.vector.tensor_add(h18_new[:, hs], n1[:, hs], d1[:, hs])
                for hf in range(2):
                    hs = slice(4 * hf, 4 * hf + 4)
                    nc.vector.tensor_add(h1T_new[:, hs], n1[:, hs], d1[:, hs])

                # ---------- D: GRU2 n gh-part (fp8 DR on old h2);
                # ghb2_j = psh2 + 256*bhh2n drained by DVE ----------
                ghb2 = tmp.tile([128, NJ, R], f16, tag="ghb")
                for j in range(NJ):
                    psh2 = psB.tile([128, R], f32, tag="g")
                    for kt in range(KP):
                        nc.tensor.matmul(psh2[:], w2hn8_sb[:, j, kt],
                                         h28[:, 2 * kt:2 * kt + 2, :],
                                         start=(kt == 0), stop=(kt == KP - 1),
                                         perf_mode=DRow)
                    nc.scalar.activation(ghb2[:, j], psh2[:], Ident,
                                         bias=bias_ap(B_N2H + j))

                # ---------- E: fco for previous step (PE filler) ----------
                if h2T_done:
                    fco_step(*h2T_done.pop())

                # ---------- F: GRU2 r/z (all fp8 DR) ----------
                r2 = gates.tile([128, NJ, R], f16, tag="rg")
                z2 = gates.tile([128, NJ, R], f16, tag="zg")
                tt2 = tmp.tile([128, NJ, R], f16, tag="tt")
                for m in range(MRZ):
                    ps = psB.tile([128, R], f32, tag="g")
                    for kt in range(KP):
                        nc.tensor.matmul(ps[:], w2h8_sb[:, m, kt],
                                         h28[:, 2 * kt:2 * kt + 2, :],
                                         start=(kt == 0), stop=False,
                                         perf_mode=DRow)
                    for kt in range(KP):
                        nc.tensor.matmul(ps[:], w2i8_sb[:, m, kt],
                                         h18_new[:, 2 * kt:2 * kt + 2, :],
                                         start=False, stop=(kt == KP - 1),
                                         perf_mode=DRow)
                    dst = r2 if m < NJ else z2
                    nc.scalar.activation(dst[:, m % NJ], ps[:], Sig,
                                         bias=bias_ap(B_RZ2 + m), scale=1.0 / WS)
                    if m == 3 or m == 7:
                        hs = slice(m - 3, m + 1)
                        nc.vector.tensor_mul(tt2[:, hs], r2[:, hs], ghb2[:, hs])

                # ---------- G: GRU2 n rest (fp16, x256-scaled weights) ------
                n2 = gates.tile([128, NJ, R], f16, tag="ng")
                for j in range(NJ):
                    psg2 = psB.tile([128, R], f32, tag="g")
                    for k in range(KH):
                        nc.tensor.matmul(psg2[:], w2in_sb[:, j, k * 128:(k + 1) * 128],
                                         h1T_new[:, k],
                                         start=(k == 0), stop=(k == KH - 1))
                    nc.vector.tensor_add(psg2[:], psg2[:], tt2[:, j])
                    nc.scalar.activation(n2[:, j], psg2[:], Tanh,
                                         bias=bias_ap(B_N2I + j), scale=1.0 / WS)
                d2 = tmp.tile([128, NJ, R], f16, tag="tt")
                nc.gpsimd.tensor_sub(d2[:], h2T[:], n2[:])
                nc.gpsimd.tensor_mul(d2[:], z2[:], d2[:])
                h2T_new = state.tile([128, NJ, R], f16, tag="h2")
                h28_new = state.tile([128, NJ, R], f8, tag="h28")
                nc.vector.tensor_add(h28_new[:], n2[:], d2[:])
                nc.vector.tensor_add(h2T_new[:], n2[:], d2[:])

                h1T, h2T = h1T_new, h2T_new
                h18, h28 = h18_new, h28_new
                h2T_done.append((h2T, s))

            fco_step(*h2T_done.pop())

    nc.compile()
    return nc


def prep_inputs(c, target, fc_init_w, fc_init_b, g1_wih, g1_whh, g1_bih, g1_bhh,
                g2_wih, g2_whh, g2_bih, g2_bhh, fco_w, fco_b):
    """Host-side shard/layout prep. Returns per-core input maps."""
    f16 = np.float16
    T = E * S
    c = np.asarray(c, np.float32)
    target = np.asarray(target, np.float32)

    g1_wih = np.asarray(g1_wih, np.float32)
    g1_whh = np.asarray(g1_whh, np.float32)
    g2_wih = np.asarray(g2_wih, np.float32)
    g2_whh = np.asarray(g2_whh, np.float32)

    def _g4(tiles):
        nm, _, w = tiles.shape
        return np.ascontiguousarray(
            tiles.reshape(nm // 4, 4, 128, w).transpose(0, 2, 1, 3)
            .reshape(nm // 4, 128, 4 * w))

    w1h8_a = _g4(_wtiles_dr(g1_whh[:2 * H], MRZ))
    w2i8_a = _g4(_wtiles_dr(g2_wih[:2 * H], MRZ))
    w2h8_a = _g4(_wtiles_dr(g2_whh[:2 * H], MRZ))
    w2hn8_a = _g4(_wtiles_dr(g2_whh[2 * H:], NJ))
    w1hn8_a = _g4(_wtiles_dr(g1_whh[2 * H:], NJ))
    # w2in: fp16, x256-scaled values (PSUM carries 256*gi2n)
    w2in_a = _g4(_wtiles(np.ascontiguousarray(g2_wih[2 * H:].T) * WS, NJ, KH))

    def _group4(tiles, ng):
        """[nm, 128, KC*128] -> [ng, 128, 4*KC*128] (4 m-tiles per chunk)"""
        return np.ascontiguousarray(
            tiles.reshape(ng, 4, 128, KC * 128).transpose(0, 2, 1, 3)
            .reshape(ng, 128, 4 * KC * 128))

    wc_a = _group4(_wtiles(g1_wih[:, :C].T, MG, KC), 6)
    wini_a = _group4(_wtiles(np.asarray(fc_init_w, np.float32).T, MI, KC), 4)

    # prev-input weights: all 3H as fp8 DR (x256)
    wp_t = g1_wih[:, C:].T                              # [130, 3072]
    wp8_a = np.zeros((128, MG, 2, 128), np.float32)
    wp8_a[:, :, 0] = (wp_t[:128] * WS).reshape(128, MG, 128)
    wp8_a[:2, :, 1] = (wp_t[128:] * WS).reshape(2, MG, 128)
    wp8_a = _q8(wp8_a)

    wfco_a = np.zeros((128, KH, 256), np.float32)
    wfco_a[:, :, :D] = np.asarray(fco_w, np.float32).T.reshape(
        KH, 128, D).transpose(1, 0, 2)
    wfco_a = np.ascontiguousarray(wfco_a.reshape(128, KH * 256)).astype(f16)

    bias = np.zeros((128, NBIAS), np.float32)
    bias[:, B_INIT:B_INIT + MI] = _bias_cols(np.asarray(fc_init_b, np.float32), MI)
    bhh1 = np.asarray(g1_bhh, np.float32)
    bih1 = np.asarray(g1_bih, np.float32)
    bhh2 = np.asarray(g2_bhh, np.float32)
    bih2 = np.asarray(g2_bih, np.float32)
    bias[:, B_N1H:B_N1H + 8] = _bias_cols(bhh1[2 * H:], 8) * WS
    bias[:, B_IH1:B_IH1 + 16] = _bias_cols((bih1 + bhh1)[:2 * H], 16) * WS
    bias[:, B_IH1 + 16:B_IH1 + 24] = _bias_cols(bih1[2 * H:], 8) * WS
    bias[:, B_RZ2:B_RZ2 + 16] = _bias_cols(bih2[:2 * H] + bhh2[:2 * H], 16)
    bias[:, B_N2H:B_N2H + 8] = _bias_cols(bhh2[2 * H:], 8) * WS
    bias[:, B_N2I:B_N2I + 8] = _bias_cols(bih2[2 * H:], 8)
    fco_b = np.asarray(fco_b, np.float32)
    bias[:, B_FCO] = fco_b[:128]
    bias[0:2, B_FCO + 1] = fco_b[128:130]

    prev_full = np.concatenate(
        [np.zeros((B, 1, D), np.float32), target[:, :T - 1]], axis=1)  # [B,T,D]

    in_maps = []
    for core in range(NCORES):
        e0 = core * EPC
        cf = c[e0:e0 + EPC].reshape(R, C)                  # [256, 512]
        # [128, KC*R]: col block k holds c-feature k-tile (2KB/partition DMA)
        cfT = np.ascontiguousarray(
            cf.T.reshape(KC, 128, R).transpose(1, 0, 2).reshape(128, KC * R)
        ).astype(f16)
        pv = prev_full[:, e0 * S:(e0 + EPC) * S]           # [B, 32, D]
        pv = pv.reshape(B, EPC, S, D).transpose(2, 1, 0, 3).reshape(S, R, D)
        pvT = np.ascontiguousarray(pv.transpose(0, 2, 1))  # [S, D, R]
        pvT8 = np.zeros((S, 128, 2, R), np.float32)
        pvT8[:, :, 0] = pvT[:, :128]
        pvT8[:, :2, 1] = pvT[:, 128:130]
        in_maps.append({
            "cflatT": cfT,
            "prevT8": _q8(pvT8),
            "w1h8": w1h8_a, "w2i8": w2i8_a, "w2h8": w2h8_a, "w2hn8": w2hn8_a,
            "w1hn8": w1hn8_a, "w2in": w2in_a,
            "wp8": wp8_a,
            "wc": wc_a, "wini": wini_a,
            "wfco": wfco_a, "biases": bias,
        })
    return in_maps


def assemble_output(results):
    """Per-core yT [S, 132, R] f32 -> full [B, T, D] f32."""
    T = E * S
    out = np.empty((B, T, D), np.float32)
    for core in range(NCORES):
        yt = results[core]["yT"]            # [S, 132, R]
        for ei in range(EPC):
            e = core * EPC + ei
            blk = yt[:, :D, ei * 128:(ei + 1) * 128]   # [S, D, 128]
            out[:, e * S:(e + 1) * S, :] = blk.transpose(2, 0, 1)
    return out


def kernel(c, target, length, batch_size, fc_init_w, fc_init_b,
           g1_wih, g1_whh, g1_bih, g1_bhh,
           g2_wih, g2_whh, g2_bih, g2_bhh, fco_w, fco_b):
    from concourse.bass_utils import run_bass_kernel_spmd

    if "nc" not in _cache:
        _cache["nc"] = build_program()
    nc = _cache["nc"]
    in_maps = prep_inputs(c, target, fc_init_w, fc_init_b,
                          g1_wih, g1_whh, g1_bih, g1_bhh,
                          g2_wih, g2_whh, g2_bih, g2_bhh, fco_w, fco_b)
    res = run_bass_kernel_spmd(nc, in_maps, list(range(NCORES)))
    return assemble_output(res.results)


# revision 40
# speedup vs baseline: 1.1027x; 1.1027x over previous
"""Trainium2 Bass kernel for nn_BottomLevelDecoderRNN.

2-layer GRU decoder, H=1024, S=16 steps, E*B = 2048 independent sequences,
data-parallel over 8 NeuronCores (R = 256 rows per core), everything kept
transposed as [feature, row].

Per-step math (per core), PE work 360 passes (DR fp8 except n2/fco):
  A rz1:  ps = DR(wp8, pv8) + DR(w1h_rz, h1_8)  [x256 PSUM]
          +cached_rz (DVE) -> sigmoid(ps/256)            [80 DR]
  B/C n1: psh = DR(w1hn8, h1_8) [x256]; tt1 = (psh+256*bhh1n)*r1 (DVE stt)
          tt1 += cached_n; psg pair = DR(wp_n, pv8); psg += tt1
          -> tanh(psg/256) paired                        [32+8 DR]
  D gh2n: psh2 = DR(w2hn8, h2_8) [x256]; ghb2 = psh2 + 256*bhh2n (DVE)
                                                         [32 DR]
  E fco(s-1) fp16                                        [16 fp16]
  F rz2:  ps = DR(w2h_rz, h2_8) + DR(w2i_rz, h1'_8) -> sigmoid(ps/256+b)
          tt2 = r2*ghb2 (DVE 4-wide)                     [128 DR]
  G n2:   psg2 = w2in16[x256] @ h1' (fp16); psg2 += tt2
          -> tanh(psg2/256 + bih2n)                      [64 fp16]

fp8 path: weights scaled x256 into e4m3 (clip 240); h states / prev cast to
e4m3 unscaled. w2in (error-sensitive) stays fp16 (x256-scaled values).
All weights SBUF-resident, DMAs spread over 4 queues. Emulated rel-err 1.39e-2.
"""
import numpy as np

E, B, C, H, D = 16, 128, 512, 1024, 130
S = 16
NCORES = 8
EPC = E // NCORES        # 2 embeddings per core
R = EPC * B              # 256 rows per core
KH = H // 128            # 8 h k-tiles
KP = KH // 2             # 4 DR k-pairs
MG = 3 * H // 128        # 24 gate m-tiles
MRZ = 2 * H // 128       # 16 rz m-tiles
NJ = H // 128            # 8 n/h tiles
KC = C // 128            # 4 c k-tiles
MI = 2 * H // 128        # 16 init m-tiles
WS = 256.0               # fp8 weight scale

# bias tile column layout ([128, NBIAS] fp32)
B_INIT = 0      # 16: fc_init_b
B_N1H = 32      # 8:  bhh1[2H:]*256
B_IH1 = 40      # 24: rz: (bih1+bhh1)*256;  n: bih1*256
B_RZ2 = 64      # 16: bih2[:2H]+bhh2[:2H]
B_N2H = 80      # 8:  bhh2[2H:]*256
B_N2I = 88      # 8:  bih2[2H:]
B_FCO = 96      # 2:  fco_b
NBIAS = 98

_cache = {}


def _wtiles(w_t, nm, nk):
    """[K, M] (w_t = W.T) -> [nm, 128, nk*128] fp16 stationary chunks."""
    Kf, Mf = w_t.shape
    assert Kf == nk * 128 and Mf == nm * 128
    return np.ascontiguousarray(
        w_t.reshape(nk, 128, nm, 128).transpose(2, 1, 0, 3).reshape(nm, 128, nk * 128)
    ).astype(np.float16)


def _q8(x):
    import ml_dtypes
    return np.clip(x, -240, 240).astype(ml_dtypes.float8_e4m3)


def _wtiles_dr(w, nm, scale=WS):
    """[nm*128, H] weight part -> [nm, 128, KP*2*128] fp8e4 DoubleRow chunks:
    chunk[m][p, kt, j, c] = (W.T)[kt*256 + j*128 + p, m*128 + c] * scale."""
    wt = np.asarray(w, np.float32).T * scale          # [H, nm*128]
    arr = wt.reshape(KP, 2, 128, nm, 128).transpose(3, 2, 0, 1, 4)
    return np.ascontiguousarray(_q8(arr).reshape(nm, 128, KP * 2 * 128))


def _bias_cols(vec, n):
    return np.ascontiguousarray(vec.reshape(n, 128).T).astype(np.float32)


def build_program():
    import concourse.tile as tile
    from concourse import bacc, mybir

    f32, f16, f8 = mybir.dt.float32, mybir.dt.float16, mybir.dt.float8e4
    Sig = mybir.ActivationFunctionType.Sigmoid
    Tanh = mybir.ActivationFunctionType.Tanh
    Ident = mybir.ActivationFunctionType.Identity
    DRow = mybir.MatmulPerfMode.DoubleRow
    ADD = mybir.AluOpType.add
    MULT = mybir.AluOpType.mult

    nc = bacc.Bacc("TRN2", target_bir_lowering=False, debug=False,
                   enable_asserts=False, num_devices=NCORES)

    def din(name, shape, dt=f16):
        return nc.dram_tensor(name, shape, dt, kind="ExternalInput").ap()

    cflatT = din("cflatT", [128, KC * R])
    prevT8 = din("prevT8", [S, 128, 2, R], f8)
    w1h8 = din("w1h8", [MRZ // 4, 128, 4 * KP * 2 * 128], f8)
    w2i8 = din("w2i8", [MRZ // 4, 128, 4 * KP * 2 * 128], f8)
    w2h8 = din("w2h8", [MRZ // 4, 128, 4 * KP * 2 * 128], f8)
    w2hn8 = din("w2hn8", [NJ // 4, 128, 4 * KP * 2 * 128], f8)
    w1hn8 = din("w1hn8", [NJ // 4, 128, 4 * KP * 2 * 128], f8)
    w2in = din("w2in", [NJ // 4, 128, 4 * KH * 128])
    wp8 = din("wp8", [128, MG, 2, 128], f8)
    wc = din("wc", [6, 128, 4 * KC * 128])      # 4 m-tiles per chunk
    wini = din("wini", [4, 128, 4 * KC * 128])
    wfco = din("wfco", [128, KH * 256])
    biases = din("biases", [128, NBIAS], f32)
    yT = nc.dram_tensor("yT", [S, 132, R], f32, kind="ExternalOutput").ap()

    with tile.TileContext(nc) as tc:
        with tc.tile_pool(name="const", bufs=1) as const, \
             tc.tile_pool(name="stream", bufs=7) as stream, \
             tc.tile_pool(name="state", bufs=2) as state, \
             tc.tile_pool(name="gates", bufs=2) as gates, \
             tc.tile_pool(name="tmp", bufs=2) as tmp, \
             tc.tile_pool(name="prevp", bufs=3) as prevp, \
             tc.tile_pool(name="outp", bufs=2) as outp, \
             tc.tile_pool(name="psA", bufs=3, space="PSUM") as psA, \
             tc.tile_pool(name="psB", bufs=5, space="PSUM") as psB:

            # ---- SBUF-resident tiles ----
            bias_sb = const.tile([128, NBIAS], f32, tag="bias")
            cfl_sb = const.tile([128, KC * R], f16, tag="cfl")
            w1h8_sb = const.tile([128, MRZ, KP, 2, 128], f8, tag="w1h8")
            w2i8_sb = const.tile([128, MRZ, KP, 2, 128], f8, tag="w2i8")
            w2h8_sb = const.tile([128, MRZ, KP, 2, 128], f8, tag="w2h8")
            w2hn8_sb = const.tile([128, NJ, KP, 2, 128], f8, tag="w2hn8")
            w1hn8_sb = const.tile([128, NJ, KP, 2, 128], f8, tag="w1hn8")
            w2in_sb = const.tile([128, NJ, KH * 128], f16, tag="w2in")
            wp8_sb = const.tile([128, MG, 2, 128], f8, tag="wp8")
            wfco_sb = const.tile([128, KH * 256], f16, tag="wfco")
            cached_sb = const.tile([128, MG, R], f16, tag="cached")

            # ---- stream-chunk tiles for init GEMMs (wini 4 + wc 6) ----
            wini_ch = [stream.tile([128, 4 * KC * 128], f16, tag="stream",
                                   name=f"wini_ch{i}") for i in range(4)]
            wc_ch = [stream.tile([128, 4 * KC * 128], f16, tag="stream",
                                 name=f"wc_ch{i}") for i in range(6)]

            # ---- all input DMAs up-front, spread over the 3 DMA-capable
            # queues (sync, scalar, gpsimd); per-queue order matches
            # first use ----
            qs_sync, qs_sc, qs_gp = nc.sync, nc.scalar, nc.gpsimd
            qs_sync.dma_start(cfl_sb[:], cflatT[:])
            qs_sc.dma_start(bias_sb[:], biases[:])
            qs_sync.dma_start(wini_ch[0][:], wini[0])
            qs_sc.dma_start(wini_ch[1][:], wini[1])
            qs_gp.dma_start(wini_ch[2][:], wini[2])
            qs_gp.dma_start(wini_ch[3][:], wini[3])
            qs_sync.dma_start(wc_ch[0][:], wc[0])
            qs_sc.dma_start(wc_ch[1][:], wc[1])
            qs_gp.dma_start(wc_ch[2][:], wc[2])
            qs_sync.dma_start(wc_ch[3][:], wc[3])
            # (wc4/wc5 feed cached_n, first used mid step 0 — issued last,
            # after the step-0 weights, so their stream-buffer waits can't
            # block the critical queue entries)
            # step-0 A/B/C weights
            qs_sc.dma_start(wp8_sb[:], wp8[:])
            for g in range(MRZ // 4):
                [qs_sync, qs_sc, qs_gp, qs_sync][g % 4].dma_start(
                    w1h8_sb[:, 4 * g:4 * g + 4], w1h8[g])
            for g in range(NJ // 4):
                [qs_gp, qs_sc][g % 2].dma_start(
                    w1hn8_sb[:, 4 * g:4 * g + 4], w1hn8[g])
            # step-0 D/F/G weights
            for g in range(NJ // 4):
                [qs_gp, qs_sync][g % 2].dma_start(
                    w2hn8_sb[:, 4 * g:4 * g + 4], w2hn8[g])
            for g in range(MRZ // 4):
                [qs_sync, qs_sc, qs_gp, qs_sync][g % 4].dma_start(
                    w2h8_sb[:, 4 * g:4 * g + 4], w2h8[g])
            for g in range(MRZ // 4):
                [qs_gp, qs_sc, qs_sc, qs_sync][g % 4].dma_start(
                    w2i8_sb[:, 4 * g:4 * g + 4], w2i8[g])
            for g in range(NJ // 4):
                [qs_sync, qs_gp][g % 2].dma_start(
                    w2in_sb[:, 4 * g:4 * g + 4], w2in[g])
            qs_sc.dma_start(wfco_sb[:], wfco[:])
            qs_sc.dma_start(wc_ch[4][:], wc[4])
            qs_gp.dma_start(wc_ch[5][:], wc[5])

            def bias_ap(col):
                return bias_sb[:, col:col + 1]

            # ---- h init: t0T = tanh(wini @ cflatT + binit) ----
            h1T = state.tile([128, NJ, R], f16, tag="h1")
            h2T = state.tile([128, NJ, R], f16, tag="h2")
            h18 = state.tile([128, NJ, R], f8, tag="h18")
            h28 = state.tile([128, NJ, R], f8, tag="h28")
            for g in range(4):
                wchunk = wini_ch[g]
                for mi in range(4):
                    m = 4 * g + mi
                    ps = psB.tile([128, R], f32, tag="g")
                    for k in range(KC):
                        nc.tensor.matmul(
                            ps[:], wchunk[:, (mi * KC + k) * 128:(mi * KC + k + 1) * 128],
                            cfl_sb[:, k * R:(k + 1) * R],
                            start=(k == 0), stop=(k == KC - 1))
                    dst = h1T if m < NJ else h2T
                    nc.scalar.activation(dst[:, m % NJ], ps[:], Tanh,
                                         bias=bias_ap(B_INIT + m))
            for j in range(NJ):
                nc.vector.tensor_copy(h18[:, j], h1T[:, j])
                nc.vector.tensor_copy(h28[:, j], h2T[:, j])

            # ---- cached = Wc @ cflatT + biases (scaled x256); rz chunks
            # (g 0-3) pre-loop, n chunks (g 4-5) deferred into step 0 ----
            def cached_chunk(g):
                wchunk = wc_ch[g]
                for mi in range(4):
                    m = 4 * g + mi
                    ps = psB.tile([128, R], f32, tag="g")
                    for k in range(KC):
                        nc.tensor.matmul(
                            ps[:], wchunk[:, (mi * KC + k) * 128:(mi * KC + k + 1) * 128],
                            cfl_sb[:, k * R:(k + 1) * R],
                            start=(k == 0), stop=(k == KC - 1))
                    nc.scalar.activation(cached_sb[:, m], ps[:], Ident,
                                         bias=bias_ap(B_IH1 + m), scale=WS)

            for g in range(4):
                cached_chunk(g)

            def fco_step(h2T_cur, s):
                for mo, msz, osz, bc in [(0, 128, 128, B_FCO), (128, 32, 2, B_FCO + 1)]:
                    ps = psB.tile([128, R], f32, tag="g")
                    for k in range(KH):
                        nc.tensor.matmul(ps[0:msz, :],
                                         wfco_sb[:, k * 256 + mo: k * 256 + mo + msz],
                                         h2T_cur[:, k],
                                         start=(k == 0), stop=(k == KH - 1))
                    ysb = outp.tile([128, R], f32, tag="y")
                    nc.scalar.activation(ysb[0:osz, :], ps[0:osz, :], Ident,
                                         bias=bias_sb[0:osz, bc:bc + 1])
                    nc.sync.dma_start(yT[s, mo:mo + osz, :], ysb[0:osz, :])

            h2T_done = []  # (h2T tile, step) pending fco

            for s in range(S):
                pv8 = prevp.tile([128, 2, R], f8, tag="pv8")
                nc.sync.dma_start(pv8[:], prevT8[s])

                # ---------- A: GRU1 r/z (fp8 DR, x256 PSUM; m-tile pairs
                # share a [128,2,R] PSUM tile and one ACT) ----------
                r1 = gates.tile([128, NJ, R], f16, tag="rg")
                z1 = gates.tile([128, NJ, R], f16, tag="zg")
                for p in range(MRZ // 2):
                    ps = psA.tile([128, 2, R], f32, tag="rz")
                    for mi in range(2):
                        m = 2 * p + mi
                        nc.tensor.matmul(ps[:, mi], wp8_sb[:, m], pv8[:],
                                         start=True, stop=False, perf_mode=DRow)
                        for kt in range(KP):
                            nc.tensor.matmul(ps[:, mi], w1h8_sb[:, m, kt],
                                             h18[:, 2 * kt:2 * kt + 2, :],
                                             start=False, stop=(kt == KP - 1),
                                             perf_mode=DRow)
                    nc.vector.tensor_add(ps[:], ps[:], cached_sb[:, 2 * p:2 * p + 2])
                    dst = r1 if p < NJ // 2 else z1
                    jj = (2 * p) % NJ
                    nc.scalar.activation(dst[:, jj:jj + 2], ps[:], Sig,
                                         bias=0.0, scale=1.0 / WS)

                if s == 0:
                    cached_chunk(4)
                    cached_chunk(5)

                # ---------- B-tail/C: GRU1 n (all fp8 DR; x256 PSUM) -------
                # psh_j = w1hn8 @ h18; tt1_j = (psh + 256*bhh1n)*r1 (DVE stt)
                tt1 = tmp.tile([128, NJ, R], f16, tag="tt")
                for j in range(NJ):
                    psh = psB.tile([128, R], f32, tag="g")
                    for kt in range(KP):
                        nc.tensor.matmul(psh[:], w1hn8_sb[:, j, kt],
                                         h18[:, 2 * kt:2 * kt + 2, :],
                                         start=(kt == 0), stop=(kt == KP - 1),
                                         perf_mode=DRow)
                    nc.vector.scalar_tensor_tensor(
                        tt1[:, j], psh[:], bias_ap(B_N1H + j), r1[:, j],
                        op0=ADD, op1=MULT)
                    if j % 4 == 3:
                        hs = slice(j - 3, j + 1)
                        nc.gpsimd.tensor_add(tt1[:, hs], tt1[:, hs],
                                             cached_sb[:, MRZ + j - 3:MRZ + j + 1])
                n1 = gates.tile([128, NJ, R], f16, tag="ng")
                for p in range(NJ // 2):
                    psg = psA.tile([128, 2, R], f32, tag="rz")
                    for mi in range(2):
                        j = 2 * p + mi
                        nc.tensor.matmul(psg[:, mi], wp8_sb[:, MRZ + j], pv8[:],
                                         start=True, stop=True, perf_mode=DRow)
                    nc.vector.tensor_add(psg[:], psg[:], tt1[:, 2 * p:2 * p + 2])
                    nc.scalar.activation(n1[:, 2 * p:2 * p + 2], psg[:], Tanh,
                                         bias=0.0, scale=1.0 / WS)
                # h1' = n1 + z1*(h1 - n1) in j-halves; fp8 result first (it
                # gates F), fp16 copy after
                d1 = tmp.tile([128, NJ, R], f16, tag="tt")
                h1T_new = state.tile([128, NJ, R], f16, tag="h1")
                h18_new = state.tile([128, NJ, R], f8, tag="h18")
                for hf in range(2):
                    hs = slice(4 * hf, 4 * hf + 4)
                    nc.vector.tensor_sub(d1[:, hs], h1T[:, hs], n1[:, hs])
                    nc.vector.tensor_mul(d1[:, hs], z1[:, hs], d1[:, hs])
                    nc.vector.tensor_add(h18_new[:, hs], n1[:, hs], d1[:, hs])
                for hf in range(2):
                    hs = slice(4 * hf, 4 * hf + 4)
                    nc.vector.tensor_add(h1T_new[:, hs], n1[:, hs], d1[:, hs])

                # ---------- D: GRU2 n gh-part (fp8 DR on old h2);
                # ghb2_j = psh2 + 256*bhh2n drained by DVE ----------
                ghb2 = tmp.tile([128, NJ, R], f16, tag="ghb")
                for j in range(NJ):
                    psh2 = psB.tile([128, R], f32, tag="g")
                    for kt in range(KP):
                        nc.tensor.matmul(psh2[:], w2hn8_sb[:, j, kt],
                                         h28[:, 2 * kt:2 * kt + 2, :],
                                         start=(kt == 0), stop=(kt == KP - 1),
                                         perf_mode=DRow)
                    nc.scalar.activation(ghb2[:, j], psh2[:], Ident,
                                         bias=bias_ap(B_N2H + j))

                # ---------- E: fco for previous step (PE filler) ----------
                if h2T_done:
                    fco_step(*h2T_done.pop())

                # ---------- F: GRU2 r/z (all fp8 DR) ----------
                r2 = gates.tile([128, NJ, R], f16, tag="rg")
                z2 = gates.tile([128, NJ, R], f16, tag="zg")
                tt2 = tmp.tile([128, NJ, R], f16, tag="tt")
                for m in range(MRZ):
                    ps = psB.tile([128, R], f32, tag="g")
                    for kt in range(KP):
                        nc.tensor.matmul(ps[:], w2h8_sb[:, m, kt],
                                         h28[:, 2 * kt:2 * kt + 2, :],
                                         start=(kt == 0), stop=False,
                                         perf_mode=DRow)
                    for kt in range(KP):
                        nc.tensor.matmul(ps[:], w2i8_sb[:, m, kt],
                                         h18_new[:, 2 * kt:2 * kt + 2, :],
                                         start=False, stop=(kt == KP - 1),
                                         perf_mode=DRow)
                    dst = r2 if m < NJ else z2
                    nc.scalar.activation(dst[:, m % NJ], ps[:], Sig,
                                         bias=bias_ap(B_RZ2 + m), scale=1.0 / WS)
                    if m == 3 or m == 7:
                        hs = slice(m - 3, m + 1)
                        nc.vector.tensor_mul(tt2[:, hs], r2[:, hs], ghb2[:, hs])

                # ---------- G: GRU2 n rest (fp16, x256-scaled weights) ------
                n2 = gates.tile([128, NJ, R], f16, tag="ng")
                for j in range(NJ):
                    psg2 = psB.tile([128, R], f32, tag="g")
                    for k in range(KH):
                        nc.tensor.matmul(psg2[:], w2in_sb[:, j, k * 128:(k + 1) * 128],
                                         h1T_new[:, k],
                                         start=(k == 0), stop=(k == KH - 1))
                    nc.vector.tensor_add(psg2[:], psg2[:], tt2[:, j])
                    nc.scalar.activation(n2[:, j], psg2[:], Tanh,
                                         bias=bias_ap(B_N2I + j), scale=1.0 / WS)
                d2 = tmp.tile([128, NJ, R], f16, tag="tt")
                nc.vector.tensor_sub(d2[:], h2T[:], n2[:])
                nc.vector.tensor_mul(d2[:], z2[:], d2[:])
                h2T_new = state.tile([128, NJ, R], f16, tag="h2")
                nc.vector.tensor_add(h2T_new[:], n2[:], d2[:])
                h28_new = state.tile([128, NJ, R], f8, tag="h28")
                nc.gpsimd.tensor_copy(h28_new[:], h2T_new[:])

                h1T, h2T = h1T_new, h2T_new
                h18, h28 = h18_new, h28_new
                h2T_done.append((h2T, s))

            fco_step(*h2T_done.pop())

    nc.compile()
    return nc


def prep_inputs(c, target, fc_init_w, fc_init_b, g1_wih, g1_whh, g1_bih, g1_bhh,
                g2_wih, g2_whh, g2_bih, g2_bhh, fco_w, fco_b):
    """Host-side shard/layout prep. Returns per-core input maps."""
    f16 = np.float16
    T = E * S
    c = np.asarray(c, np.float32)
    target = np.asarray(target, np.float32)

    g1_wih = np.asarray(g1_wih, np.float32)
    g1_whh = np.asarray(g1_whh, np.float32)
    g2_wih = np.asarray(g2_wih, np.float32)
    g2_whh = np.asarray(g2_whh, np.float32)

    def _g4(tiles):
        nm, _, w = tiles.shape
        return np.ascontiguousarray(
            tiles.reshape(nm // 4, 4, 128, w).transpose(0, 2, 1, 3)
            .reshape(nm // 4, 128, 4 * w))

    w1h8_a = _g4(_wtiles_dr(g1_whh[:2 * H], MRZ))
    w2i8_a = _g4(_wtiles_dr(g2_wih[:2 * H], MRZ))
    w2h8_a = _g4(_wtiles_dr(g2_whh[:2 * H], MRZ))
    w2hn8_a = _g4(_wtiles_dr(g2_whh[2 * H:], NJ))
    w1hn8_a = _g4(_wtiles_dr(g1_whh[2 * H:], NJ))
    # w2in: fp16, x256-scaled values (PSUM carries 256*gi2n)
    w2in_a = _g4(_wtiles(np.ascontiguousarray(g2_wih[2 * H:].T) * WS, NJ, KH))

    def _group4(tiles, ng):
        """[nm, 128, KC*128] -> [ng, 128, 4*KC*128] (4 m-tiles per chunk)"""
        return np.ascontiguousarray(
            tiles.reshape(ng, 4, 128, KC * 128).transpose(0, 2, 1, 3)
            .reshape(ng, 128, 4 * KC * 128))

    wc_a = _group4(_wtiles(g1_wih[:, :C].T, MG, KC), 6)
    wini_a = _group4(_wtiles(np.asarray(fc_init_w, np.float32).T, MI, KC), 4)

    # prev-input weights: all 3H as fp8 DR (x256)
    wp_t = g1_wih[:, C:].T                              # [130, 3072]
    wp8_a = np.zeros((128, MG, 2, 128), np.float32)
    wp8_a[:, :, 0] = (wp_t[:128] * WS).reshape(128, MG, 128)
    wp8_a[:2, :, 1] = (wp_t[128:] * WS).reshape(2, MG, 128)
    wp8_a = _q8(wp8_a)

    wfco_a = np.zeros((128, KH, 256), np.float32)
    wfco_a[:, :, :D] = np.asarray(fco_w, np.float32).T.reshape(
        KH, 128, D).transpose(1, 0, 2)
    wfco_a = np.ascontiguousarray(wfco_a.reshape(128, KH * 256)).astype(f16)

    bias = np.zeros((128, NBIAS), np.float32)
    bias[:, B_INIT:B_INIT + MI] = _bias_cols(np.asarray(fc_init_b, np.float32), MI)
    bhh1 = np.asarray(g1_bhh, np.float32)
    bih1 = np.asarray(g1_bih, np.float32)
    bhh2 = np.asarray(g2_bhh, np.float32)
    bih2 = np.asarray(g2_bih, np.float32)
    bias[:, B_N1H:B_N1H + 8] = _bias_cols(bhh1[2 * H:], 8) * WS
    bias[:, B_IH1:B_IH1 + 16] = _bias_cols((bih1 + bhh1)[:2 * H], 16) * WS
    bias[:, B_IH1 + 16:B_IH1 + 24] = _bias_cols(bih1[2 * H:], 8) * WS
    bias[:, B_RZ2:B_RZ2 + 16] = _bias_cols(bih2[:2 * H] + bhh2[:2 * H], 16)
    bias[:, B_N2H:B_N2H + 8] = _bias_cols(bhh2[2 * H:], 8) * WS
    bias[:, B_N2I:B_N2I + 8] = _bias_cols(bih2[2 * H:], 8)
    fco_b = np.asarray(fco_b, np.float32)
    bias[:, B_FCO] = fco_b[:128]
    bias[0:2, B_FCO + 1] = fco_b[128:130]

    prev_full = np.concatenate(
        [np.zeros((B, 1, D), np.float32), target[:, :T - 1]], axis=1)  # [B,T,D]

    in_maps = []
    for core in range(NCORES):
        e0 = core * EPC
        cf = c[e0:e0 + EPC].reshape(R, C)                  # [256, 512]
        # [128, KC*R]: col block k holds c-feature k-tile (2KB/partition DMA)
        cfT = np.ascontiguousarray(
            cf.T.reshape(KC, 128, R).transpose(1, 0, 2).reshape(128, KC * R)
        ).astype(f16)
        pv = prev_full[:, e0 * S:(e0 + EPC) * S]           # [B, 32, D]
        pv = pv.reshape(B, EPC, S, D).transpose(2, 1, 0, 3).reshape(S, R, D)
        pvT = np.ascontiguousarray(pv.transpose(0, 2, 1))  # [S, D, R]
        pvT8 = np.zeros((S, 128, 2, R), np.float32)
        pvT8[:, :, 0] = pvT[:, :128]
        pvT8[:, :2, 1] = pvT[:, 128:130]
        in_maps.append({
            "cflatT": cfT,
            "prevT8": _q8(pvT8),
            "w1h8": w1h8_a, "w2i8": w2i8_a, "w2h8": w2h8_a, "w2hn8": w2hn8_a,
            "w1hn8": w1hn8_a, "w2in": w2in_a,
            "wp8": wp8_a,
            "wc": wc_a, "wini": wini_a,
            "wfco": wfco_a, "biases": bias,
        })
    return in_maps


def assemble_output(results):
    """Per-core yT [S, 132, R] f32 -> full [B, T, D] f32."""
    T = E * S
    out = np.empty((B, T, D), np.float32)
    for core in range(NCORES):
        yt = results[core]["yT"]            # [S, 132, R]
        for ei in range(EPC):
            e = core * EPC + ei
            blk = yt[:, :D, ei * 128:(ei + 1) * 128]   # [S, D, 128]
            out[:, e * S:(e + 1) * S, :] = blk.transpose(2, 0, 1)
    return out


def kernel(c, target, length, batch_size, fc_init_w, fc_init_b,
           g1_wih, g1_whh, g1_bih, g1_bhh,
           g2_wih, g2_whh, g2_bih, g2_bhh, fco_w, fco_b):
    from concourse.bass_utils import run_bass_kernel_spmd

    if "nc" not in _cache:
        _cache["nc"] = build_program()
    nc = _cache["nc"]
    in_maps = prep_inputs(c, target, fc_init_w, fc_init_b,
                          g1_wih, g1_whh, g1_bih, g1_bhh,
                          g2_wih, g2_whh, g2_bih, g2_bhh, fco_w, fco_b)
    res = run_bass_kernel_spmd(nc, in_maps, list(range(NCORES)))
    return assemble_output(res.results)
